# revision 30
# baseline (speedup 1.0000x reference)
"""Trainium2 Bass kernel: fused multi-head causal self-attention block.

Computes, for x:(B,S,H), W_qkv:(3H,H), b_qkv:(3H,), W_out:(H,H), b_out:(H,):
    qkv = x @ W_qkv.T + b_qkv ; split into q,k,v heads (NH heads, D=H/NH)
    out = softmax(causal(q k^T / sqrt(D))) v   ; merge heads
    return out @ W_out.T + b_out

Sharding over 8 NeuronCores: DP(2 batches) x TP(4 head-groups).
Core c handles batch b=c//4, head group g=c%4 (heads 4g..4g+3).

Schedule (per core): V-projection first, then HEAD-MAJOR q/k projection +
attention so each head's AllGather is issued as early as possible; the four
AllGathers then run back-to-back on the collective cores, hidden behind the
remaining heads' projection/attention/out-proj work.  Out-proj for head l
is interleaved into attention of head l+2.

All matmuls are fp16 (measured end-to-end max-rel err ~3e-4; fp8 variants
measured at 1.2e-2..5.8e-2 and rejected).  PSUM accumulation is fp32.
"""

import math

import numpy as np

import concourse.bass as bass
import concourse.mybir as mybir
import concourse.tile as tile
from concourse import bacc
from concourse.tile_rust import add_dep_helper
from concourse.bass_utils import run_bass_kernel_spmd

FP = mybir.dt.float32
F16 = mybir.dt.float16

# Full-size problem constants.
B, S, H, NH = 2, 2048, 2048, 16
D = 128
NCORES = 8
GROUPS = 4                  # head-groups per batch (TP degree)
REPLICA_GROUPS = [[0, 1, 2, 3], [4, 5, 6, 7]]

TRACE = False
LAST_EXEC_NS = None
LAST_RESULTS = None


def build_nc(s=S, h=H, nh=NH, reps=1, ag=True):
    """Build the SPMD Bass program (identical on all 8 cores)."""
    nl = nh // GROUPS           # local heads per core
    dg = nl * D                 # per-core slice of the head dim

    nc = bacc.Bacc(
        "TRN2",
        target_bir_lowering=False,
        debug=False,
        enable_asserts=False,
        num_devices=NCORES,
    )

    # ---- I/O -----------------------------------------------------------
    xT_d = nc.dram_tensor("xT", [h, s], F16, kind="ExternalInput")
    wq_d = nc.dram_tensor("wq", [h, dg], F16, kind="ExternalInput")
    wk_d = nc.dram_tensor("wk", [h, dg], F16, kind="ExternalInput")
    wv_d = nc.dram_tensor("wv", [h, dg], F16, kind="ExternalInput")
    wo_d = nc.dram_tensor("wo", [h, dg], F16, kind="ExternalInput")
    bq_d = nc.dram_tensor("bq", [128, nl], FP, kind="ExternalInput")
    bk_d = nc.dram_tensor("bk", [128, nl], FP, kind="ExternalInput")
    bv_d = nc.dram_tensor("bv", [128, dg], FP, kind="ExternalInput")
    bo_d = nc.dram_tensor("bo", [128, dg], FP, kind="ExternalInput")
    mask_d = nc.dram_tensor("mask", [128, 896], F16, kind="ExternalInput")
    ones_d = nc.dram_tensor("ones", [128, 128], F16, kind="ExternalInput")
    out_d = nc.dram_tensor("out", [s, dg], FP, kind="ExternalOutput")

    with tile.TileContext(nc) as tc:
        with tc.tile_pool(name="const", bufs=1) as constp, \
             tc.tile_pool(name="wts", bufs=1) as wtsp:
            mask_sb = constp.tile([128, 896], F16)
            nc.sync.dma_start(mask_sb[:], mask_d[:])
            ones_sb = constp.tile([128, 128], F16)
            nc.sync.dma_start(ones_sb[:], ones_d[:])
            onesf_sb = constp.tile([1, 128], F16)
            nc.vector.memset(onesf_sb[:], 1.0)
            zrow_sb = constp.tile([1, 512], F16)
            nc.vector.memset(zrow_sb[:], 0.0)
            bq_sb = constp.tile([128, nl], FP)
            nc.sync.dma_start(bq_sb[:], bq_d[:])
            bk_sb = constp.tile([128, nl], FP)
            nc.sync.dma_start(bk_sb[:], bk_d[:])
            bv_sb = constp.tile([128, dg], FP)
            nc.sync.dma_start(bv_sb[:], bv_d[:])
            bo_sb = constp.tile([128, dg], FP)
            nc.sync.dma_start(bo_sb[:], bo_d[:])

            ones_col = ones_sb[:, 0:1]        # [128,1] lhsT for denominator sum
            ones_row = onesf_sb[0:1, :]       # [1,128] lhsT for partition-broadcast

            # weight tiles (loads scheduled inside _emit_body for pacing)
            wq_sb = [wtsp.tile([128, 4, dg], F16, tag=f"wq{hb}", name=f"wq{hb}")
                     for hb in range(4)]
            wk_sb = [wtsp.tile([128, 4, dg], F16, tag=f"wk{hb}", name=f"wk{hb}")
                     for hb in range(4)]
            wv_sb = [wtsp.tile([128, 4, dg], F16, tag=f"wv{hb}", name=f"wv{hb}")
                     for hb in range(4)]
            wo_sb = [wtsp.tile([128, 4, dg], F16, tag=f"wo{hb}", name=f"wo{hb}")
                     for hb in range(4)]

            for _rep in range(reps):
                _emit_body(nc, tc, s, h, nh,
                           xT_d, wq_d, wk_d, wv_d, wo_d, out_d,
                           wq_sb, wk_sb, wv_sb, wo_sb,
                           bq_sb, bk_sb, bv_sb, bo_sb,
                           mask_sb, ones_col, ones_row, zrow_sb, ag)

    nc.compile()
    return nc


def _emit_body(nc, tc, s, h, nh,
               xT_d, wq_d, wk_d, wv_d, wo_d, out_d,
               wq_sb, wk_sb, wv_sb, wo_sb,
               bq_sb, bk_sb, bv_sb, bo_sb,
               mask_sb, ones_col, ones_row, zrow_sb, ag=True):
    nl = nh // GROUPS
    dg = nl * D
    hc = h // 128               # 128-row contraction chunks
    hb_n = hc // 4
    sq = s // 512               # 512-wide strips
    st_n = s // 128             # 128-row s tiles
    scale = 1.0 / math.sqrt(D)

    def load_w(w_sb, w_d, hb):
        rows = slice(512 * hb, 512 * hb + 512)
        nc.scalar.dma_start(w_sb[hb][:],
                            w_d[rows, :].rearrange("(c p) d -> p c d", p=128))

    with tc.tile_pool(name="qkv", bufs=1) as qkvp:
        qT = [qkvp.tile([128, s], F16, tag=f"qT{t}", name=f"qT{t}") for t in range(nl)]
        kT = [qkvp.tile([128, s], F16, tag=f"kT{t}", name=f"kT{t}") for t in range(nl)]
        vv = [qkvp.tile([128, dg], F16, tag=f"v{t}", name=f"v{t}") for t in range(st_n)]

        # ---- head-major: per head [V-proj strips (head 0 only) +
        #      Q/K-proj strip + attention strip], AllGather per head -------
        with tc.tile_pool(name="xA", bufs=4) as xap, \
             tc.tile_pool(name="xV", bufs=3) as xvp, \
             tc.tile_pool(name="psV", bufs=1, space="PSUM") as psV, \
             tc.tile_pool(name="etp", bufs=8) as etp, \
             tc.tile_pool(name="atp", bufs=3) as atp, \
             tc.tile_pool(name="rbp", bufs=2) as rbp, \
             tc.tile_pool(name="oaccp", bufs=1) as oaccp, \
             tc.tile_pool(name="atsp", bufs=4) as atsp, \
             tc.tile_pool(name="outp", bufs=2) as outp, \
             tc.tile_pool(name="dramp", bufs=1, space="DRAM") as dramp, \
             tc.tile_pool(name="psS", bufs=2, space="PSUM") as psS, \
             tc.tile_pool(name="psAV", bufs=1, space="PSUM") as psAV, \
             tc.tile_pool(name="psDN", bufs=1, space="PSUM") as psDN, \
             tc.tile_pool(name="psO", bufs=1, space="PSUM") as psO:

            oacc = [oaccp.tile([128, dg], FP, tag=f"oacc{sti}", name=f"oacc{sti}")
                    for sti in range(st_n)]
            agouts = []

            def a2_strip(i):
                """V-projection for tokens [256*i, 256*i+256)."""
                cs = slice(256 * i, 256 * i + 256)
                psv = [psV.tile([128, dg], FP, tag=f"psv{sti}", name=f"psv{sti}")
                       for sti in range(2)]
                for hb in range(hb_n):
                    xch2 = xvp.tile([128, 4, 256], F16, tag="xch2", name="xch2")
                    if i == 0:
                        load_w(wv_sb, wv_d, hb)
                    elif i == 1:
                        load_w(wq_sb, wq_d, hb)
                        load_w(wk_sb, wk_d, hb)
                    nc.scalar.dma_start(
                        xch2[:],
                        xT_d[512 * hb:512 * hb + 512, cs].rearrange("(c p) t -> p c t", p=128))
                    for c in range(4):
                        hh = 4 * hb + c
                        for sti in range(2):
                            nc.tensor.matmul(
                                psv[sti][:],
                                xch2[:, c, 128 * sti:128 * sti + 128],
                                wv_sb[hb][:, c, :],
                                start=(hh == 0), stop=(hh == hc - 1),
                            )
                for sti in range(2):
                    nc.vector.tensor_add(vv[2 * i + sti][:], psv[sti][:], bv_sb[:])

            def a1_strip(l, strip):
                """Q^T/K^T projection strip for local head l (psum via psS pool)."""
                if True:
                    cs = slice(512 * strip, 512 * strip + 512)
                    psq = psS.tile([128, 512], FP, tag="ps_s", name="ps_s")
                    psk = psS.tile([128, 512], FP, tag="ps_s", name="ps_s")
                    # all q matmuls first so the qT drain (needed by every
                    # attention score of this strip) overlaps the k matmuls
                    xchs = []
                    for hb in range(hb_n):
                        xch = xap.tile([128, 4, 512], F16, tag="xch", name="xch")
                        if l == 0 and strip == 0:
                            load_w(wo_sb, wo_d, hb)
                        nc.scalar.dma_start(
                            xch[:],
                            xT_d[512 * hb:512 * hb + 512, cs].rearrange("(c p) t -> p c t", p=128))
                        xchs.append(xch)
                        for c in range(4):
                            hh = 4 * hb + c
                            nc.tensor.matmul(
                                psq[:],
                                wq_sb[hb][:, c, 128 * l:128 * l + 128],
                                xch[:, c, :],
                                start=(hh == 0), stop=(hh == hc - 1),
                            )
                    nc.scalar.activation(
                        qT[l][:, cs], psq[:],
                        mybir.ActivationFunctionType.Identity,
                        bias=bq_sb[:, l:l + 1],
                    )
                    for hb in range(hb_n):
                        for c in range(4):
                            hh = 4 * hb + c
                            nc.tensor.matmul(
                                psk[:],
                                wk_sb[hb][:, c, 128 * l:128 * l + 128],
                                xchs[hb][:, c, :],
                                start=(hh == 0), stop=(hh == hc - 1),
                            )
                    nc.scalar.activation(
                        kT[l][:, cs], psk[:],
                        mybir.ActivationFunctionType.Identity,
                        bias=bk_sb[:, l:l + 1],
                    )

            def outproj_steps(l, pin_after=None):
                """Generator: 16 sti-steps of out-projection for head l."""
                last = (l == nl - 1)
                for sti in range(st_n):
                    rs = slice(128 * sti, 128 * sti + 128)
                    at4 = atsp.tile([128, 4, 128], F16, tag="at4", name="at4")
                    at4_dma = nc.scalar.dma_start(
                        at4[:],
                        agouts[l][:, rs].rearrange("(r p) t -> p r t", p=128))
                    if pin_after is not None:
                        # ordering-only edge: keep this load from being
                        # scheduler-hoisted ahead of earlier heads' work on
                        # the ACT queue (it waits on its AllGather and would
                        # head-of-line block the queue there)
                        add_dep_helper(at4_dma.ins, pin_after.ins, False,
                                       reason="pin out-proj after collective")
                    # rotate the accumulator across psO plus the two psV
                    # banks (idle after head 0's V prelude; same byte size)
                    # so consecutive steps never serialize on one psum bank
                    if sti % 3 == 0:
                        ps_o = psO.tile([128, dg], FP, tag="ps_o", name="ps_o")
                    else:
                        ps_o = psV.tile([128, dg], FP, tag=f"psv{sti % 3 - 1}",
                                        name="ps_o")
                    for r in range(4):
                        nc.tensor.matmul(
                            ps_o[:], at4[:, r, :], wo_sb[l][:, r, :],
                            start=(r == 0), stop=(r == 3),
                        )
                    if l == 0:
                        nc.vector.tensor_add(oacc[sti][:], ps_o[:], bo_sb[:])
                    elif not last:
                        nc.vector.tensor_add(oacc[sti][:], ps_o[:], oacc[sti][:])
                    else:
                        ob = outp.tile([128, dg], FP, tag="ob", name="ob")
                        nc.vector.tensor_add(ob[:], ps_o[:], oacc[sti][:])
                        nc.sync.dma_start(out_d[rs, :], ob[:])
                    yield

            def attention_head(l, op_steps, prelude=None):
                agin = dramp.tile([128, s], F16, tag=f"agin{l}", name=f"agin{l}")
                pend = [None]

                def flush_epilogue():
                    if pend[0] is not None:
                        pend[0]()
                        pend[0] = None
                for qs in range(sq):
                    if prelude is not None:
                        prelude(qs)
                    a1_strip(l, qs)
                    qsl = slice(512 * qs, 512 * qs + 512)
                    ps_av = psAV.tile([128, 512], FP, tag="ps_av", name="ps_av")
                    ps_dn = psDN.tile([1, 512], FP, tag="ps_dn", name="ps_dn")
                    nf = 4 * qs            # full (unmasked) 512-wide k-tiles
                    # diagonal 512x512 block processed as [128,128] sub-tiles:
                    # row dk only covers q-chunks j >= dk, so 6/16 fully
                    # masked sub-tiles are skipped entirely
                    units = list(range(nf)) + [4096 + dk for dk in range(4)]
                    nu = len(units)

                    ets = {}

                    def emit_score(u):
                        ps_s = psS.tile([128, 512], FP, tag="ps_s", name="ps_s")
                        et = etp.tile([128, 512], F16, tag="et", name="et")
                        if u < 4096:
                            nc.tensor.matmul(
                                ps_s[:],
                                kT[l][:, 128 * u:128 * u + 128],
                                qT[l][:, qsl],
                                start=True, stop=True,
                            )
                            nc.scalar.activation(
                                et[:], ps_s[:],
                                mybir.ActivationFunctionType.Exp,
                                scale=scale,
                            )
                        else:
                            dk = u - 4096
                            kt = nf + dk
                            for j in range(dk, 4):
                                nc.tensor.matmul(
                                    ps_s[:, 128 * j:128 * j + 128],
                                    kT[l][:, 128 * kt:128 * kt + 128],
                                    qT[l][:, 512 * qs + 128 * j:512 * qs + 128 * j + 128],
                                    start=True, stop=True,
                                )
                            nc.scalar.activation(
                                et[:, 128 * dk:512], ps_s[:, 128 * dk:512],
                                mybir.ActivationFunctionType.Exp,
                                scale=scale,
                            )
                            nc.vector.tensor_mul(
                                et[:, 128 * dk:128 * dk + 128],
                                et[:, 128 * dk:128 * dk + 128],
                                mask_sb[:, 384:512])
                        ets[u] = et

                    def emit_dnav(u):
                        et = ets.pop(u)
                        if u < 4096:
                            nc.tensor.matmul(
                                ps_dn[:], ones_col, et[:],
                                start=False, stop=False,
                                skip_group_check=True,
                            )
                            nc.tensor.matmul(
                                ps_av[:],
                                vv[u][:, 128 * l:128 * l + 128],
                                et[:],
                                start=False, stop=False,
                                skip_group_check=True,
                            )
                        else:
                            dk = u - 4096
                            kt = nf + dk
                            for j in range(dk, 4):
                                cj = slice(128 * j, 128 * j + 128)
                                st = False
                                sp = (j == dk)
                                nc.tensor.matmul(
                                    ps_dn[0:1, cj], ones_col, et[:, cj],
                                    start=st, stop=sp,
                                    skip_group_check=True,
                                )
                                nc.tensor.matmul(
                                    ps_av[:, cj],
                                    vv[kt][:, 128 * l:128 * l + 128],
                                    et[:, cj],
                                    start=st, stop=sp,
                                    skip_group_check=True,
                                )

                    # software-pipelined: scores lead dn/av by 2 units so the
                    # exp (ACT) + causal-mask (DVE) chain never stalls the PE.
                    # The PREVIOUS strip's softmax epilogue is emitted after
                    # this strip's prologue scores so its DVE/ACT round-trip
                    # hides behind queued PE work instead of stalling it.
                    emit_score(units[0])
                    if nu > 1:
                        emit_score(units[1])
                    flush_epilogue()
                    # zero-contribution rank-1 matmuls initialize the full
                    # psum regions so every later write can accumulate
                    # (per-range start=True zeroing proved unsafe on HW)
                    nc.tensor.matmul(ps_dn[:], zrow_sb[0:1, 0:1], zrow_sb[:],
                                     start=True, stop=False,
                                     skip_group_check=True)
                    nc.tensor.matmul(ps_av[:], ones_row, zrow_sb[:],
                                     start=True, stop=False,
                                     skip_group_check=True)
                    for j in range(nu):
                        if j + 2 < nu:
                            emit_score(units[j + 2])
                        emit_dnav(units[j])

                    def epilogue(ps_av=ps_av, ps_dn=ps_dn, qsl=qsl):
                        # normalize: an = ps_av * (1/denom); denom broadcast
                        # across partitions via a cheap fp16 rank-1 matmul
                        rb32 = rbp.tile([1, 512], FP, tag="rb32", name="rb32")
                        nc.vector.reciprocal(rb32[:], ps_dn[:])
                        rb16 = rbp.tile([1, 512], F16, tag="rb16", name="rb16")
                        nc.scalar.activation(rb16[:], rb32[:],
                                             mybir.ActivationFunctionType.Copy)
                        ps_rb = psDN.tile([128, 512], FP, tag="ps_rb",
                                          name="ps_rb", bufs=1)
                        nc.tensor.matmul(ps_rb[:], ones_row, rb16[:],
                                         start=True, stop=True)
                        rbs = rbp.tile([128, 512], F16, tag="rbs", name="rbs")
                        nc.scalar.activation(rbs[:], ps_rb[:],
                                             mybir.ActivationFunctionType.Copy)
                        an = atp.tile([128, 512], F16, tag="an", name="an")
                        nc.vector.tensor_mul(an[:], ps_av[:], rbs[:])
                        nc.sync.dma_start(agin[:, qsl], an[:])
                    pend[0] = epilogue
                flush_epilogue()
                # AllGather this head's A^T across the batch group
                agout = dramp.tile([512, s], F16, tag=f"agout{l}", name=f"agout{l}")
                if ag:
                    cc = nc.gpsimd.collective_compute(
                        "AllGather",
                        mybir.AluOpType.bypass,
                        replica_groups=REPLICA_GROUPS,
                        ins=[agin.opt()],
                        outs=[agout.opt()],
                    )
                else:
                    cc = nc.sync.dma_start(agout[0:128, :], agin[:])
                agouts.append(agout)
                return cc

            # out-proj for head l-2 emitted AFTER collective(l): its AllGather
            # is complete by then, so its at4 loads never head-of-line block
            # the DMA queues ahead of later agin writes / collectives.
            ccs = []
            for l in range(nl):
                prelude = (lambda qs: (a2_strip(2 * qs), a2_strip(2 * qs + 1))) \
                    if l == 0 else None
                ccs.append(attention_head(l, None, prelude))
                if l >= 2:
                    for _ in outproj_steps(l - 2, pin_after=ccs[l]):
                        pass
            for l in range(nl - 2, nl):
                for _ in outproj_steps(l, pin_after=ccs[nl - 1]):
                    pass


def make_inputs(x, W_qkv, b_qkv, W_out, b_out, s=S, h=H, nh=NH):
    """Host-side sharding: per-core input dicts."""
    nl = nh // GROUPS
    dg = nl * D
    x = np.asarray(x, dtype=np.float32)
    W_qkv = np.asarray(W_qkv, dtype=np.float32)
    b_qkv = np.asarray(b_qkv, dtype=np.float32)
    W_out = np.asarray(W_out, dtype=np.float32)
    b_out = np.asarray(b_out, dtype=np.float32)

    # causal staircase master mask: mask[i, u] = 1 iff u >= i + 384
    uu = np.arange(896)[None, :]
    ii = np.arange(128)[:, None]
    mask = (uu >= ii + 384).astype(np.float16)
    ones = np.ones((128, 128), dtype=np.float16)

    WoT = W_out.T  # [h (d-in), h (n-out)]
    in_maps = []
    for c in range(NCORES):
        b, g = divmod(c, GROUPS)
        xT = np.ascontiguousarray(x[b].T.astype(np.float16))    # [h, s]
        wq = np.ascontiguousarray(W_qkv[dg * g:dg * (g + 1), :].T.astype(np.float16))
        wk = np.ascontiguousarray(W_qkv[h + dg * g:h + dg * (g + 1), :].T.astype(np.float16))
        wv = np.ascontiguousarray(W_qkv[2 * h + dg * g:2 * h + dg * (g + 1), :].T.astype(np.float16))
        bq = np.ascontiguousarray(
            b_qkv[dg * g:dg * (g + 1)].reshape(nl, 128).T)      # [128, nl]
        bk = np.ascontiguousarray(
            b_qkv[h + dg * g:h + dg * (g + 1)].reshape(nl, 128).T)
        bv = np.tile(b_qkv[2 * h + dg * g:2 * h + dg * (g + 1)][None, :], (128, 1))
        bo = np.tile(b_out[dg * g:dg * (g + 1)][None, :], (128, 1))
        # W_out^T rows permuted to the AllGather d-order:
        # ci = l*4 + r  ->  global head 4r + l (within this batch group)
        blocks = []
        for l in range(nl):
            for r in range(GROUPS):
                hh = nl * r + l  # head held as local-head l by group-rank r
                blocks.append(WoT[D * hh:D * (hh + 1), dg * g:dg * (g + 1)])
        wo = np.ascontiguousarray(
            np.concatenate(blocks, axis=0).astype(np.float16))  # [h, dg] fp16
        in_maps.append({
            "xT": xT, "wq": wq, "wk": wk, "wv": wv, "wo": wo,
            "bq": bq, "bk": bk,
            "bv": np.ascontiguousarray(bv), "bo": np.ascontiguousarray(bo),
            "mask": mask, "ones": ones,
        })
    return in_maps


_NC_CACHE = {}


def _get_nc(key=(S, H, NH)):
    if key not in _NC_CACHE:
        _NC_CACHE[key] = build_nc(*key)
    return _NC_CACHE[key]


def kernel(x, W_qkv, b_qkv, W_out, b_out):
    global LAST_EXEC_NS, LAST_RESULTS
    nc = _get_nc()
    in_maps = make_inputs(x, W_qkv, b_qkv, W_out, b_out)
    res = run_bass_kernel_spmd(
        nc, in_maps, core_ids=list(range(NCORES)), trace=TRACE)
    LAST_EXEC_NS = res.exec_time_ns
    LAST_RESULTS = res
    nl = NH // GROUPS
    dg = nl * D
    out = np.empty((B, S, H), dtype=np.float32)
    for c in range(NCORES):
        b, g = divmod(c, GROUPS)
        out[b, :, dg * g:dg * (g + 1)] = res.results[c]["out"]
    return out
